# revision 1
# baseline (speedup 1.0000x reference)
"""GRU cell kernel for Trainium2, data-parallel across 8 NeuronCores.

Reference computation (per batch row):
    concat = [h_prev, x]                       # [B, 2048]
    z = sigmoid(concat @ W_z.T + b_z)          # [B, 1024]
    r = sigmoid(concat @ W_r.T + b_r)
    h_tilde = tanh([r*h_prev, x] @ W_h.T + b_h)
    h = (1-z)*h_prev + z*h_tilde

Sharding: batch dim (8192) split 1024/core; weights replicated.
Layout on device is feature-major ([feature, batch]) so the batch is the
matmul moving dimension (N=512 per PSUM bank) and the hidden units are the
PSUM partition dim. Host transposes in/out.

mm_dtype selects the matmul precision:
  f32r — TF32 PE mode, fp32 storage (rel err ~1e-4)
  bf16 — bf16 operands (weights/acts), fp32 h_prev kept for elementwise
  f32  — true fp32 matmuls (4x slower on PE)
"""

import numpy as np

import concourse.bacc as bacc
import concourse.bass as bass
import concourse.mybir as mybir
import concourse.tile as tile
from concourse import bass_utils

P = 128
B = 8192
I = 1024
H = 1024
K = I + H            # 2048 contraction
NCORES = 8
BS = B // NCORES     # 1024 batch rows per core
MT = H // P          # 8 m-tiles (hidden units)
KT = K // P          # 16 k-chunks
NFREE = 512          # matmul moving free dim (one PSUM bank of fp32)
NT = BS // NFREE     # 2 n-tiles per core

F32 = mybir.dt.float32
F32R = mybir.dt.float32r
BF16 = mybir.dt.bfloat16

AF = mybir.ActivationFunctionType


def build_kernel(mm_dtype: str = "f32r"):
    """Build the per-core Bass kernel. Returns compiled nc."""
    mdt = {"f32r": F32R, "f32": F32, "bf16": BF16}[mm_dtype]
    bf16 = mm_dtype == "bf16"
    nc = bacc.Bacc("TRN2", target_bir_lowering=False, debug=False)

    # DRAM I/O (per-core shapes). Matmul-feeding tensors carry the matmul
    # dtype (f32r is the same bits as f32 on the host side).
    xT = nc.dram_tensor("xT", [I, BS], mdt, kind="ExternalInput").ap()
    hT = nc.dram_tensor("hT", [H, BS], mdt, kind="ExternalInput").ap()
    if bf16:  # separate fp32 copy of h_prev for the elementwise path
        hTf = nc.dram_tensor("hTf", [H, BS], F32, kind="ExternalInput").ap()
    Wz = nc.dram_tensor("Wz", [MT, P, K], mdt, kind="ExternalInput").ap()
    Wr = nc.dram_tensor("Wr", [MT, P, K], mdt, kind="ExternalInput").ap()
    Wh = nc.dram_tensor("Wh", [MT, P, K], mdt, kind="ExternalInput").ap()
    bz = nc.dram_tensor("bz", [P, MT], F32, kind="ExternalInput").ap()
    br = nc.dram_tensor("br", [P, MT], F32, kind="ExternalInput").ap()
    bh = nc.dram_tensor("bh", [P, MT], F32, kind="ExternalInput").ap()
    out = nc.dram_tensor("out", [H, BS], F32, kind="ExternalOutput").ap()

    def ew(ap):
        """fp32 view of an f32r AP for elementwise use (same bits)."""
        return ap.bitcast(F32) if mdt == F32R else ap

    with tile.TileContext(nc) as tc:
        with (
            tc.tile_pool(name="acts", bufs=1) as acts,
            tc.tile_pool(name="gates", bufs=1) as gates,
            tc.tile_pool(name="wpool", bufs=5) as wpool,
            tc.tile_pool(name="opool", bufs=6) as opool,
            tc.tile_pool(name="ppool", bufs=8, space="PSUM") as ppool,
        ):
            # Biases first — they gate the first sigmoid (psum recycling).
            # Scalar HWDGE ring so they don't sit behind the act loads.
            bz_sb = acts.tile([P, MT], F32)
            br_sb = acts.tile([P, MT], F32)
            bh_sb = acts.tile([P, MT], F32)
            nc.scalar.dma_start(br_sb[:], br)
            nc.scalar.dma_start(bz_sb[:], bz)
            nc.scalar.dma_start(bh_sb[:], bh)

            # First two weight tiles go at the HEAD of the sync HWDGE ring:
            # within a ring DMAs drain FIFO, so they get full SDMA bandwidth
            # before the act loads start, instead of a round-robin share.
            # (The SWDGE queue used for the remaining tiles takes ~6us to
            # spin up anyway.)
            w_pre = {("r", i): wpool.tile([P, K], mdt, tag="w", name=f"wf{i}")
                     for i in range(6)}
            nc.sync.dma_start(w_pre[("r", 0)][:], Wr[0])
            nc.scalar.dma_start(w_pre[("r", 1)][:], Wr[1])
            nc.scalar.dma_start(w_pre[("r", 3)][:], Wr[3])

            # Pre-warm the ACT sigmoid/tanh table set during the DMA fill so
            # the first real sigmoid doesn't pay the ~2.7us ACT_TABLE_LOAD.
            # Reads its own uninitialized tile — no DMA dependency, result
            # discarded — so it cannot stall the scalar ring's weight DMAs.
            warm = acts.tile([P, 1], F32)
            nc.scalar.activation(warm[:], warm[:], AF.Sigmoid)

            # Persistent activations, feature-major: [p, ko, batch]
            xT_sb = acts.tile([P, I // P, BS], mdt)
            hT_sb = acts.tile([P, H // P, BS], mdt)
            hTf_sb = (acts.tile([P, H // P, BS], F32, name="hTf_sb")
                      if bf16 else None)
            # Load per (tensor, batch-half), n=0 halves first, so the first
            # PSUM groups (n=0) are gated on half the act bytes. One DMA per
            # half: each dma_start costs ~600ns of descriptor-gen serialized
            # on the sync sequencer, so many small chunk loads would delay
            # the bytes the first PSUM group needs. Weight DMAs ride the
            # idle GpSimd SWDGE queue so they don't serialize with act
            # loads or compute issue.
            xT_r = xT.rearrange("(ko p) b -> p ko b", p=P)
            hT_r = hT.rearrange("(ko p) b -> p ko b", p=P)
            hTf_r = hTf.rearrange("(ko p) b -> p ko b", p=P) if bf16 else None
            # Interleave the next R-gate weight tiles into the sync FIFO so
            # they drain right AFTER the bytes the first PSUM groups need,
            # instead of contending with them from the gpsimd ring.
            for n in range(NT):
                ns = slice(n * NFREE, (n + 1) * NFREE)
                nc.sync.dma_start(hT_sb[:, :, ns], hT_r[:, :, ns])
                if n == 0:
                    # w2 drains before xT-n0: the interleaved ramp consumes
                    # (w0..w3, hT-n0) first and must not head-of-line block.
                    nc.sync.dma_start(w_pre[("r", 2)][:], Wr[2])
                nc.sync.dma_start(xT_sb[:, :, ns], xT_r[:, :, ns])
                if n == 1:
                    nc.sync.dma_start(w_pre[("r", 4)][:], Wr[4])
            if bf16:
                for half in range(2):
                    ks = slice(half * 4, (half + 1) * 4)
                    nc.sync.dma_start(hTf_sb[:, ks, :], hTf_r[:, ks, :])
            nc.sync.dma_start(w_pre[("r", 5)][:], Wr[5])

            # Gate results, feature-major
            z_sb = gates.tile([P, MT, BS], F32)
            rh_sb = gates.tile([P, MT, BS], mdt)

            def hprev_ew(mt, ns):
                """fp32-precision h_prev slice for elementwise use."""
                if bf16:
                    return hTf_sb[:, mt, ns]
                return ew(hT_sb[:, mt, ns])

            def rhs_chunk(k, base, width, stage):
                """Moving operand [128, width] for contraction chunk k."""
                if k < H // P:
                    src = rh_sb if stage == "h" else hT_sb
                    return src[:, k, base:base + width]
                return xT_sb[:, k - H // P, base:base + width]

            def gate(stage, Wd, b_sb):
                if stage == "r":
                    # Interleaved ramp: open 4 PSUM groups (mt0-3, n=0),
                    # k-outer across them, so the PE runs 32 matmuls on the
                    # already-arrived h_prev half while the x half of the
                    # batch (and later weights) are still in flight.
                    NG = 4
                    ws = [w_pre[("r", g)] for g in range(NG)]
                    pss = [ppool.tile([P, NFREE], F32, tag="ps",
                                      name=f"psri{g}") for g in range(NG)]
                    for k in range(KT):
                        for g in range(NG):
                            nc.tensor.matmul(
                                pss[g], ws[g][:, k * P:(k + 1) * P],
                                rhs_chunk(k, 0, NFREE, stage),
                                start=(k == 0), stop=(k == KT - 1))
                    for g in range(NG):
                        ns0 = slice(0, NFREE)
                        r_tmp = opool.tile([P, NFREE], F32, tag="rt",
                                           name=f"rti{g}")
                        nc.scalar.activation(
                            r_tmp, pss[g], AF.Sigmoid, bias=b_sb[:, g:g + 1])
                        nc.vector.tensor_mul(
                            rh_sb[:, g, ns0], r_tmp, hprev_ew(g, ns0))
                    plan = ([(mt, 1) for mt in range(NG)]
                            + [(mt, n) for mt in range(NG, MT)
                               for n in range(NT)])
                else:
                    plan = [(mt, n) for mt in range(MT) for n in range(NT)]
                for mt, n in plan:
                    w_sb = w_pre.get((stage, mt))
                    if w_sb is None:
                        w_sb = wpool.tile([P, K], mdt, tag="w")
                        nc.gpsimd.dma_start(w_sb[:], Wd[mt])
                        w_pre[(stage, mt)] = w_sb
                    if True:
                        # Split the very last group so its activation+combine
                        # +store chain pipelines instead of sitting fully
                        # exposed after the final matmul.
                        last = stage == "h" and mt == MT - 1 and n == NT - 1
                        nsub = 2 if last else 1
                        width = NFREE // nsub
                        for s in range(nsub):
                            base = n * NFREE + s * width
                            ps = ppool.tile([P, width], F32, tag="ps",
                                            name=f"ps{mt}_{n}_{s}")
                            for k in range(KT):
                                nc.tensor.matmul(
                                    ps,
                                    w_sb[:, k * P:(k + 1) * P],
                                    rhs_chunk(k, base, width, stage),
                                    start=(k == 0),
                                    stop=(k == KT - 1),
                                )
                            ns = slice(base, base + width)
                            bias = b_sb[:, mt:mt + 1]
                            if stage == "r":
                                # r -> rh = r * h_prev, in matmul dtype
                                r_tmp = opool.tile([P, width], F32, tag="rt")
                                nc.scalar.activation(
                                    r_tmp, ps, AF.Sigmoid, bias=bias)
                                nc.vector.tensor_mul(
                                    rh_sb[:, mt, ns], r_tmp, hprev_ew(mt, ns))
                            elif stage == "z":
                                nc.scalar.activation(
                                    z_sb[:, mt, ns], ps, AF.Sigmoid, bias=bias)
                            else:  # h = h_prev + z*(tanh(pre) - h_prev)
                                ht = opool.tile([P, width], F32, tag="ht",
                                                name=f"ht{mt}_{n}_{s}")
                                nc.scalar.activation(
                                    ht, ps, AF.Tanh, bias=bias)
                                nc.vector.tensor_sub(ht, ht, hprev_ew(mt, ns))
                                nc.vector.tensor_mul(ht, ht, z_sb[:, mt, ns])
                                nc.vector.tensor_add(ht, ht, hprev_ew(mt, ns))
                                nc.sync.dma_start(
                                    out[mt * P:(mt + 1) * P, ns], ht)

            gate("r", Wr, br_sb)
            gate("z", Wz, bz_sb)
            gate("h", Wh, bh_sb)

    nc.compile()
    return nc


def _prep_inputs(x, h_prev, W_z, b_z, W_r, b_r, W_h, b_h, np_dtype=np.float32):
    """Host-side relayout: feature-major activations, m-tiled weights."""
    bf16 = np_dtype != np.float32

    def prep_w(W):
        # want w[mt, p, ko*128+m] = W[mt*128+m, ko*128+p]
        W4 = W.reshape(MT, P, KT, P)          # [mt, m, ko, p]
        return np.ascontiguousarray(
            W4.transpose(0, 3, 2, 1)).reshape(MT, P, K).astype(np_dtype)

    def prep_b(b):
        return np.ascontiguousarray(b.reshape(MT, P).T)

    xT = np.ascontiguousarray(x.T).astype(np_dtype)       # [I, B]
    hTf = np.ascontiguousarray(h_prev.T)                  # [H, B] f32
    hT = hTf.astype(np_dtype)
    shared = {
        "Wz": prep_w(W_z), "Wr": prep_w(W_r), "Wh": prep_w(W_h),
        "bz": prep_b(b_z), "br": prep_b(b_r), "bh": prep_b(b_h),
    }
    in_maps = []
    for c in range(NCORES):
        bs = slice(c * BS, (c + 1) * BS)
        m = dict(shared)
        m["xT"] = np.ascontiguousarray(xT[:, bs])
        m["hT"] = np.ascontiguousarray(hT[:, bs])
        if bf16:
            m["hTf"] = np.ascontiguousarray(hTf[:, bs])
        in_maps.append(m)
    return in_maps


def run(inputs, mm_dtype="bf16", trace=False, **run_kwargs):
    """Compile + run on 8 cores. Returns (output [B,H] f32, BassKernelResults)."""
    import ml_dtypes
    nc = build_kernel(mm_dtype)
    np_dtype = ml_dtypes.bfloat16 if mm_dtype == "bf16" else np.float32
    in_maps = _prep_inputs(**inputs, np_dtype=np_dtype)
    res = bass_utils.run_bass_kernel_spmd(
        nc, in_maps, core_ids=list(range(NCORES)), trace=trace, **run_kwargs)
    outT = np.concatenate(
        [res.results[c]["out"] for c in range(NCORES)], axis=1)  # [H, B]
    return np.ascontiguousarray(outT.T), res


def kernel(**inputs) -> np.ndarray:
    import time as _time
    try:
        out, _ = run(inputs)
    except Exception:
        # The axon-tunneled device occasionally reports a transient
        # "unrecoverable" state right after a crashed session; a fresh
        # attempt after a short pause recovers.
        _time.sleep(15)
        out, _ = run(inputs)
    return out



# revision 4
# speedup vs baseline: 1.5169x; 1.5169x over previous
"""GRU cell kernel for Trainium2, data-parallel across 8 NeuronCores.

Reference computation (per batch row):
    concat = [h_prev, x]                       # [B, 2048]
    z = sigmoid(concat @ W_z.T + b_z)          # [B, 1024]
    r = sigmoid(concat @ W_r.T + b_r)
    h_tilde = tanh([r*h_prev, x] @ W_h.T + b_h)
    h = (1-z)*h_prev + z*h_tilde

Sharding: batch dim (8192) split 1024/core; weights replicated.
Layout on device is feature-major ([feature, batch]); batch is the matmul
moving dimension, hidden units the PSUM partition dim. Host transposes.

Precision: matmuls run in fp8-e4m3 with perf_mode=DoubleRow (2 contraction
rows per PE cell -> 2x bf16 throughput; the 0.5 cyc/row path in the cost
model). Weights are scaled by 512 on the host so |w|<=11.3 sits in e4m3's
normal range (raw |w|<=0.022 would be subnormal); the scale is undone for
free via the activation instruction's scale operand. Elementwise runs in
bf16 (2x/4x DVE modes); h_prev is kept in bf16 for the elementwise path.

mode:
  fp8h  - all three gates fp8-DR.           (sim rel_fro ~1.74e-2)
  split - r/z fp8-DR; h-gate h-part fp8-DR over r*h_prev, x-part bf16.
                                            (sim rel_fro ~1.21e-2)
"""

import numpy as np

import concourse.bacc as bacc
import concourse.bass as bass
import concourse.mybir as mybir
import concourse.tile as tile
from concourse import bass_utils

P = 128
B = 8192
I = 1024
H = 1024
K = I + H            # 2048 contraction
NCORES = 8
BS = B // NCORES     # 1024 batch rows per core
MT = H // P          # 8 m-tiles (hidden units)
KT = K // P          # 16 k-chunks of 128
KK = K // (2 * P)    # 8 double-chunks of 256 (DoubleRow)
NFREE = 512          # moving free dim (one PSUM bank of fp32)
NT = BS // NFREE     # 2 n-tiles per core
WS = 512.0           # host-side weight scale for fp8 range

F32 = mybir.dt.float32
BF16 = mybir.dt.bfloat16
F8 = mybir.dt.float8e4

AF = mybir.ActivationFunctionType
DR = mybir.MatmulPerfMode.DoubleRow


def build_kernel(mode: str = "fp8h"):
    """Build the per-core Bass kernel. Returns compiled nc."""
    assert mode in ("fp8h", "split")
    split = mode == "split"
    nc = bacc.Bacc("TRN2", target_bir_lowering=False, debug=False)

    # DRAM I/O (per-core shapes)
    x8 = nc.dram_tensor("x8", [I, BS], F8, kind="ExternalInput").ap()
    h8 = nc.dram_tensor("h8", [H, BS], F8, kind="ExternalInput").ap()
    hb = nc.dram_tensor("hb", [H, BS], BF16, kind="ExternalInput").ap()
    Wr = nc.dram_tensor("Wr", [MT, P, K], F8, kind="ExternalInput").ap()
    Wz = nc.dram_tensor("Wz", [MT, P, K], F8, kind="ExternalInput").ap()
    if split:
        xb = nc.dram_tensor("xb", [I, BS], BF16, kind="ExternalInput").ap()
        Whh = nc.dram_tensor("Whh", [MT, P, H], F8, kind="ExternalInput").ap()
        Whx = nc.dram_tensor("Whx", [MT, P, I], BF16,
                             kind="ExternalInput").ap()
    else:
        Wh = nc.dram_tensor("Wh", [MT, P, K], F8, kind="ExternalInput").ap()
    bz = nc.dram_tensor("bz", [P, MT], F32, kind="ExternalInput").ap()
    br = nc.dram_tensor("br", [P, MT], F32, kind="ExternalInput").ap()
    bh = nc.dram_tensor("bh", [P, MT], F32, kind="ExternalInput").ap()
    out = nc.dram_tensor("out", [H, BS], BF16, kind="ExternalOutput").ap()

    with tile.TileContext(nc) as tc:
        with (
            tc.tile_pool(name="acts", bufs=1) as acts,
            tc.tile_pool(name="gates", bufs=1) as gates,
            tc.tile_pool(name="wpool", bufs=1) as wpool,
            tc.tile_pool(name="opool", bufs=8) as opool,
            tc.tile_pool(name="ppool", bufs=8, space="PSUM") as ppool,
        ):
            # Biases first on the scalar HWDGE ring (gate the first sigmoid).
            bz_sb = acts.tile([P, MT], F32)
            br_sb = acts.tile([P, MT], F32)
            bh_sb = acts.tile([P, MT], F32)
            nc.scalar.dma_start(br_sb[:], br)
            nc.scalar.dma_start(bz_sb[:], bz)
            nc.scalar.dma_start(bh_sb[:], bh)

            # Weight tiles, [P, KT, P] so [:, 2k:2k+2, :] is a DoubleRow
            # stationary operand [128, 2, 128].
            wr_sb = [wpool.tile([P, KT, P], F8, name=f"wr{m}")
                     for m in range(MT)]
            wz_sb = [wpool.tile([P, KT, P], F8, name=f"wz{m}")
                     for m in range(MT)]
            if split:
                whh_sb = [wpool.tile([P, KT // 2, P], F8, name=f"whh{m}")
                          for m in range(MT)]
                whx_sb = [wpool.tile([P, I], BF16, name=f"whx{m}")
                          for m in range(MT)]
            else:
                wh_sb = [wpool.tile([P, KT, P], F8, name=f"wh{m}")
                         for m in range(MT)]

            # Pre-warm the ACT sigmoid/tanh tables during the DMA fill.
            warm = acts.tile([P, 1], F32)
            nc.vector.memset(warm[:], 0.0)
            nc.scalar.activation(warm[:], warm[:], AF.Sigmoid)

            # Persistent activations, feature-major: [p, ko, batch]
            x8_sb = acts.tile([P, I // P, BS], F8)
            h8_sb = acts.tile([P, H // P, BS], F8)
            hb_sb = acts.tile([P, H // P, BS], BF16)
            xb_sb = (acts.tile([P, I // P, BS], BF16, name="xb_sb")
                     if split else None)
            x8_r = x8.rearrange("(ko p) b -> p ko b", p=P)
            h8_r = h8.rearrange("(ko p) b -> p ko b", p=P)
            hb_r = hb.rearrange("(ko p) b -> p ko b", p=P)
            xb_r = xb.rearrange("(ko p) b -> p ko b", p=P) if split else None

            # Head DMA schedule. Sync ring (FIFO drain) carries exactly what
            # the r-gate ramp needs, in consumption order; scalar ring takes
            # the next-needed weights + the bf16 elementwise copy; the gpsimd
            # SWDGE queue (slow ~6us spin-up) takes everything later-needed.
            n0 = slice(0, NFREE)
            n1 = slice(NFREE, BS)
            nc.sync.dma_start(wr_sb[0][:], Wr[0])
            nc.sync.dma_start(h8_sb[:, :, n0], h8_r[:, :, n0])
            nc.sync.dma_start(h8_sb[:, :, n1], h8_r[:, :, n1])
            nc.sync.dma_start(x8_sb[:, :, n0], x8_r[:, :, n0])
            nc.sync.dma_start(x8_sb[:, :, n1], x8_r[:, :, n1])
            nc.scalar.dma_start(wr_sb[1][:], Wr[1])
            nc.scalar.dma_start(wr_sb[2][:], Wr[2])
            nc.scalar.dma_start(wr_sb[3][:], Wr[3])
            for m in range(2):
                ks = slice(m * 4, (m + 1) * 4)
                nc.scalar.dma_start(hb_sb[:, ks, :], hb_r[:, ks, :])
            for m in range(4, MT):
                nc.gpsimd.dma_start(wr_sb[m][:], Wr[m])
            for m in range(MT):
                nc.gpsimd.dma_start(wz_sb[m][:], Wz[m])
            if split:
                for m in range(2):
                    ks = slice(m * 4, (m + 1) * 4)
                    nc.scalar.dma_start(xb_sb[:, ks, :], xb_r[:, ks, :])
                for m in range(MT):
                    nc.gpsimd.dma_start(whh_sb[m][:], Whh[m])
                for m in range(MT):
                    nc.gpsimd.dma_start(whx_sb[m][:], Whx[m])
            else:
                for m in range(MT):
                    nc.gpsimd.dma_start(wh_sb[m][:], Wh[m])

            # Gate results, feature-major
            z_sb = gates.tile([P, MT, BS], BF16)
            rh_sb = gates.tile([P, MT, BS], F8)

            def rz_rhs(kk, ns):
                """fp8 moving operand [128,2,n] for concat chunk kk."""
                if kk < KK // 2:
                    return h8_sb[:, 2 * kk:2 * kk + 2, ns]
                c = kk - KK // 2
                return x8_sb[:, 2 * c:2 * c + 2, ns]

            def h_rhs(kk, ns):
                """fp8 moving operand for the h-gate ([r*h_prev, x])."""
                if kk < KK // 2:
                    return rh_sb[:, 2 * kk:2 * kk + 2, ns]
                c = kk - KK // 2
                return x8_sb[:, 2 * c:2 * c + 2, ns]

            def finish(stage, mt, n, ps, b_sb, width=NFREE, sub=0):
                """PSUM -> activation -> elementwise -> (store)."""
                base = n * NFREE + sub * width
                ns = slice(base, base + width)
                bias = b_sb[:, mt:mt + 1]
                if stage == "r":
                    r_tmp = opool.tile([P, width], BF16, tag="rt")
                    nc.scalar.activation(r_tmp, ps, AF.Sigmoid, bias=bias,
                                         scale=1.0 / WS)
                    nc.vector.tensor_mul(
                        rh_sb[:, mt, ns], r_tmp, hb_sb[:, mt, ns])
                elif stage == "z":
                    nc.scalar.activation(z_sb[:, mt, ns], ps, AF.Sigmoid,
                                         bias=bias, scale=1.0 / WS)
                else:  # h = h_prev + z*(h_tilde - h_prev)
                    ht = opool.tile([P, width], BF16, tag="ht")
                    nc.scalar.activation(ht, ps, AF.Tanh, bias=bias,
                                         scale=1.0 / WS)
                    nc.vector.tensor_sub(ht, ht, hb_sb[:, mt, ns])
                    nc.vector.tensor_mul(ht, ht, z_sb[:, mt, ns])
                    nc.vector.tensor_add(ht, ht, hb_sb[:, mt, ns])
                    nc.sync.dma_start(out[mt * P:(mt + 1) * P, ns], ht)

            def dr_group(stage, w_sb, rhs, mts, b_sb, n_vals=(0, 1),
                         kk_n_outer=False):
                """k-outer DoubleRow accumulation for a set of m-tiles.

                Weight chunk (mt,kk) is loaded once and reused for every n in
                n_vals, so the 256-col DR LDWEIGHTS hides under >=2 matmuls.
                kk_n_outer emits (n outermost per kk) single-MM chains for the
                cold-start ramp where LDW hides under slow cold matmuls and
                DMA arrival order (h n0 -> h n1 -> x n0 -> x n1) must match.
                """
                ps = {(mt, n): ppool.tile([P, NFREE], F32, tag="ps",
                                          name=f"ps_{stage}{mt}_{n}")
                      for mt in mts for n in n_vals}
                if kk_n_outer:
                    order = ([(kk, mt, n) for half in range(2)
                              for n in n_vals for kk in
                              range(half * KK // 2, (half + 1) * KK // 2)
                              for mt in mts])
                else:
                    order = [(kk, mt, n) for kk in range(KK)
                             for mt in mts for n in n_vals]
                for kk, mt, n in order:
                    nc.tensor.matmul(
                        ps[(mt, n)],
                        w_sb[mt][:, 2 * kk:2 * kk + 2, :],
                        rhs(kk, slice(n * NFREE, (n + 1) * NFREE)),
                        start=(kk == 0), stop=(kk == KK - 1), perf_mode=DR)
                for mt in mts:
                    for n in n_vals:
                        finish(stage, mt, n, ps[(mt, n)], b_sb)

            # --- r gate ---
            # Ramp: mt0-1, both batch halves, n-outer-per-half single-MM
            # chains (cold PE hides per-MM LDW; ordering tracks DMA arrival).
            dr_group("r", wr_sb, rz_rhs, (0, 1), br_sb, kk_n_outer=True)
            # Warm: k-outer with weight reuse.
            dr_group("r", wr_sb, rz_rhs, (2, 3), br_sb)
            dr_group("r", wr_sb, rz_rhs, (4, 5, 6, 7), br_sb)

            # --- z gate ---
            dr_group("z", wz_sb, rz_rhs, (0, 1, 2, 3), bz_sb)
            dr_group("z", wz_sb, rz_rhs, (4, 5, 6, 7), bz_sb)

            # --- h gate ---
            if not split:
                dr_group("h", wh_sb, h_rhs, (0, 1, 2, 3), bh_sb)
                dr_group("h", wh_sb, h_rhs, (4, 5), bh_sb)
                # Split the last two groups' tail so activation+combine+store
                # pipelines instead of sitting fully exposed at the end.
                for mt in (6, 7):
                    ps = {n: ppool.tile([P, NFREE], F32, tag="ps",
                                        name=f"ps_h{mt}_{n}")
                          for n in range(NT)}
                    for kk in range(KK):
                        for n in range(NT):
                            nc.tensor.matmul(
                                ps[n], wh_sb[mt][:, 2 * kk:2 * kk + 2, :],
                                h_rhs(kk, slice(n * NFREE, (n + 1) * NFREE)),
                                start=(kk == 0), stop=(kk == KK - 1),
                                perf_mode=DR)
                    for n in range(NT):
                        for s in range(2):
                            w2 = NFREE // 2
                            finish("h", mt, n,
                                   ps[n][:, s * w2:(s + 1) * w2],
                                   bh_sb, width=w2, sub=s)
            else:
                # bf16 x-part (8 plain chunks) + fp8-DR h-part (4 dchunks)
                # accumulated into the same PSUM group. Whx is scaled like
                # the fp8 weights so one activation scale undoes both.
                for mts in ((0, 1, 2, 3), (4, 5), (6, 7)):
                    ps = {(mt, n): ppool.tile([P, NFREE], F32, tag="ps",
                                              name=f"ps_h{mt}_{n}")
                          for mt in mts for n in range(NT)}
                    for kc in range(KT // 2):
                        for mt in mts:
                            for n in range(NT):
                                nsl = slice(n * NFREE, (n + 1) * NFREE)
                                nc.tensor.matmul(
                                    ps[(mt, n)],
                                    whx_sb[mt][:, kc * P:(kc + 1) * P],
                                    xb_sb[:, kc, nsl],
                                    start=(kc == 0), stop=False)
                    for kk in range(KK // 2):
                        for mt in mts:
                            for n in range(NT):
                                nsl = slice(n * NFREE, (n + 1) * NFREE)
                                nc.tensor.matmul(
                                    ps[(mt, n)],
                                    whh_sb[mt][:, 2 * kk:2 * kk + 2, :],
                                    rh_sb[:, 2 * kk:2 * kk + 2, nsl],
                                    start=False, stop=(kk == KK // 2 - 1),
                                    perf_mode=DR)
                    last = mts == (6, 7)
                    for mt in mts:
                        for n in range(NT):
                            if last:
                                for s in range(2):
                                    w2 = NFREE // 2
                                    finish("h", mt, n,
                                           ps[(mt, n)][:, s * w2:(s + 1) * w2],
                                           bh_sb, width=w2, sub=s)
                            else:
                                finish("h", mt, n, ps[(mt, n)], bh_sb)

    nc.compile()
    return nc


def _prep_inputs(x, h_prev, W_z, b_z, W_r, b_r, W_h, b_h, mode="fp8h"):
    """Host-side relayout: feature-major activations, m-tiled weights."""
    import ml_dtypes
    F8NP = ml_dtypes.float8_e4m3fn
    BFNP = ml_dtypes.bfloat16
    split = mode == "split"

    def prep_w(W, dt):
        # w[mt, p, ko*128+m] = W[mt*128+m, ko*128+p], scaled for fp8 range
        MTl, Kl = W.shape[0] // P, W.shape[1]
        W4 = (W * WS).reshape(MTl, P, Kl // P, P)      # [mt, m, ko, p]
        return np.ascontiguousarray(
            W4.transpose(0, 3, 2, 1)).reshape(MTl, P, Kl).astype(dt)

    def prep_b(b):
        return np.ascontiguousarray(b.reshape(MT, P).T)

    xT = np.ascontiguousarray(x.T)                         # [I, B] f32
    hT = np.ascontiguousarray(h_prev.T)                    # [H, B] f32
    x8 = xT.astype(F8NP)
    h8 = hT.astype(F8NP)
    hb = hT.astype(BFNP)
    shared = {
        "Wr": prep_w(W_r, F8NP), "Wz": prep_w(W_z, F8NP),
        "bz": prep_b(b_z), "br": prep_b(b_r), "bh": prep_b(b_h),
    }
    if split:
        shared["Whh"] = prep_w(W_h[:, :H], F8NP)
        shared["Whx"] = prep_w(W_h[:, H:], BFNP)
        xbf = xT.astype(BFNP)
    else:
        shared["Wh"] = prep_w(W_h, F8NP)
    in_maps = []
    for c in range(NCORES):
        bs = slice(c * BS, (c + 1) * BS)
        m = dict(shared)
        m["x8"] = np.ascontiguousarray(x8[:, bs])
        m["h8"] = np.ascontiguousarray(h8[:, bs])
        m["hb"] = np.ascontiguousarray(hb[:, bs])
        if split:
            m["xb"] = np.ascontiguousarray(xbf[:, bs])
        in_maps.append(m)
    return in_maps


def run(inputs, mode="fp8h", trace=False, **run_kwargs):
    """Compile + run on 8 cores. Returns (output [B,H] f32, results)."""
    run_kwargs.pop("mm_dtype", None)
    nc = build_kernel(mode)
    in_maps = _prep_inputs(**inputs, mode=mode)
    res = bass_utils.run_bass_kernel_spmd(
        nc, in_maps, core_ids=list(range(NCORES)), trace=trace, **run_kwargs)
    outT = np.concatenate(
        [res.results[c]["out"] for c in range(NCORES)], axis=1)  # [H, B] bf16
    return np.ascontiguousarray(outT.T).astype(np.float32), res


def kernel(**inputs) -> np.ndarray:
    import time as _time
    try:
        out, _ = run(inputs)
    except Exception:
        # The axon-tunneled device occasionally reports a transient
        # "unrecoverable" state right after a crashed session; a fresh
        # attempt after a short pause recovers.
        _time.sleep(15)
        out, _ = run(inputs)
    return out


# revision 5
# speedup vs baseline: 1.7373x; 1.1453x over previous
"""GRU cell kernel for Trainium2, data-parallel across 8 NeuronCores.

Reference computation (per batch row):
    concat = [h_prev, x]                       # [B, 2048]
    z = sigmoid(concat @ W_z.T + b_z)          # [B, 1024]
    r = sigmoid(concat @ W_r.T + b_r)
    h_tilde = tanh([r*h_prev, x] @ W_h.T + b_h)
    h = (1-z)*h_prev + z*h_tilde

Sharding: batch dim (8192) split 1024/core; weights replicated.
Layout on device is feature-major; batch is the matmul moving dimension,
hidden units the PSUM partition dim. Host transposes in/out.

Matmuls run in fp8-e4m3 with perf_mode=DoubleRow (2 contraction rows per
PE cell). The PE moving port feeds 2 bytes/partition/cycle, so a DR
matmul streams a [256 x 512-batch] contraction chunk in ~512 cycles --
2x the flops of bf16 per cycle; measured ~216ns/MM = ~155 TF/s, the fp8
roofline. Weights are host-scaled by 512 so |w|<=11.3 sits in e4m3's
normal range (raw |w|<=0.022 is subnormal); the activation instruction's
scale operand undoes it for free.

Activations are host-swizzled to [partition, batch-half, feature-chunk,
512] so every DMA moves 4KB-contiguous runs per partition (128
descriptors/transfer instead of 1024 512B ones).

mode:
  fp8h  - all three gates fp8-DR.           (HW rel_fro ~1.76e-2)
  split - r/z fp8-DR; h-gate h-part fp8-DR over r*h_prev, x-part bf16.
                                            (sim rel_fro ~1.25e-2)
"""

import numpy as np

import concourse.bacc as bacc
import concourse.bass as bass
import concourse.mybir as mybir
import concourse.tile as tile
from concourse import bass_utils

P = 128
B = 8192
I = 1024
H = 1024
K = I + H            # 2048 contraction
NCORES = 8
BS = B // NCORES     # 1024 batch rows per core
MT = H // P          # 8 m-tiles (hidden units)
KT = K // P          # 16 k-chunks of 128
KK = K // (2 * P)    # 8 double-chunks of 256 (DoubleRow)
NFREE = 512          # moving free dim (one PSUM bank of fp32)
NT = BS // NFREE     # 2 n-tiles per core
KO = 8               # feature chunks per 1024-feature tensor
WS = 512.0           # host-side weight scale for fp8 range

F32 = mybir.dt.float32
BF16 = mybir.dt.bfloat16
F8 = mybir.dt.float8e4

AF = mybir.ActivationFunctionType
DR = mybir.MatmulPerfMode.DoubleRow


def build_kernel(mode: str = "fp8h"):
    """Build the per-core Bass kernel. Returns compiled nc."""
    assert mode in ("fp8h", "split")
    split = mode == "split"
    nc = bacc.Bacc("TRN2", target_bir_lowering=False, debug=False)

    # DRAM I/O (per-core shapes). Activations are pre-swizzled on the host
    # to [P, NT*KO*NFREE] so each partition's bytes are contiguous.
    AW = NT * KO * NFREE
    x8 = nc.dram_tensor("x8", [P, AW], F8, kind="ExternalInput").ap()
    h8 = nc.dram_tensor("h8", [P, AW], F8, kind="ExternalInput").ap()
    hb = nc.dram_tensor("hb", [P, AW], BF16, kind="ExternalInput").ap()
    Wr = nc.dram_tensor("Wr", [MT, P, K], F8, kind="ExternalInput").ap()
    Wz = nc.dram_tensor("Wz", [MT, P, K], F8, kind="ExternalInput").ap()
    if split:
        xb = nc.dram_tensor("xb", [P, AW], BF16, kind="ExternalInput").ap()
        Whh = nc.dram_tensor("Whh", [MT, P, H], F8, kind="ExternalInput").ap()
        Whx = nc.dram_tensor("Whx", [MT, P, I], BF16,
                             kind="ExternalInput").ap()
    else:
        Wh = nc.dram_tensor("Wh", [MT, P, K], F8, kind="ExternalInput").ap()
    bz = nc.dram_tensor("bz", [P, MT], F32, kind="ExternalInput").ap()
    br = nc.dram_tensor("br", [P, MT], F32, kind="ExternalInput").ap()
    bh = nc.dram_tensor("bh", [P, MT], F32, kind="ExternalInput").ap()
    out = nc.dram_tensor("out", [H, BS], BF16, kind="ExternalOutput").ap()

    with tile.TileContext(nc) as tc:
        with (
            tc.tile_pool(name="acts", bufs=1) as acts,
            tc.tile_pool(name="gates", bufs=1) as gates,
            tc.tile_pool(name="wpool", bufs=1) as wpool,
            tc.tile_pool(name="opool", bufs=10) as opool,
            tc.tile_pool(name="ppool", bufs=8, space="PSUM") as ppool,
        ):
            bz_sb = acts.tile([P, MT], F32)
            br_sb = acts.tile([P, MT], F32)
            bh_sb = acts.tile([P, MT], F32)

            # Weight tiles, [P, KT, P]: [:, 2k:2k+2, :] is a DoubleRow
            # stationary operand [128, 2, 128].
            wr_sb = [wpool.tile([P, KT, P], F8, name=f"wr{m}")
                     for m in range(MT)]
            wz_sb = [wpool.tile([P, KT, P], F8, name=f"wz{m}")
                     for m in range(MT)]
            if split:
                whh_sb = [wpool.tile([P, KT // 2, P], F8, name=f"whh{m}")
                          for m in range(MT)]
                whx_sb = [wpool.tile([P, I], BF16, name=f"whx{m}")
                          for m in range(MT)]
            else:
                wh_sb = [wpool.tile([P, KT, P], F8, name=f"wh{m}")
                         for m in range(MT)]

            # Pre-warm the ACT sigmoid table during the DMA fill.
            warm = acts.tile([P, 1], F32)
            nc.vector.memset(warm[:], 0.0)
            nc.scalar.activation(warm[:], warm[:], AF.Sigmoid)

            # Persistent activations: [p, n-half, ko, bw]
            x8_sb = acts.tile([P, NT, KO, NFREE], F8)
            h8_sb = acts.tile([P, NT, KO, NFREE], F8)
            hb_sb = acts.tile([P, NT, KO, NFREE], BF16)
            xb_sb = (acts.tile([P, NT, KO, NFREE], BF16, name="xb_sb")
                     if split else None)

            def half(dram, n):
                return dram[:, n * KO * NFREE:(n + 1) * KO * NFREE]

            # Head DMA schedule, in consumption order of the (mt, n) chains:
            # mt0/n0 consumes h8-n0 chunks 0-1 first, so that sliver leads
            # the sync ring; wr0 rides the scalar ring in parallel (ahead of
            # the ACT table loads the framework hoists there). The gpsimd
            # SWDGE queue (~1us extra latency, otherwise idle) takes
            # everything needed later than ~12us.
            q0 = 2 * NFREE  # first two feature-chunks of a half
            nc.scalar.dma_start(wr_sb[0][:], Wr[0])
            nc.sync.dma_start(h8_sb[:, 0, 0:2, :], h8[:, 0:q0])
            nc.sync.dma_start(h8_sb[:, 0, 2:, :], h8[:, q0:KO * NFREE])
            nc.sync.dma_start(x8_sb[:, 0], half(x8, 0))
            nc.sync.dma_start(h8_sb[:, 1], half(h8, 1))
            nc.sync.dma_start(x8_sb[:, 1], half(x8, 1))
            nc.scalar.dma_start(br_sb[:], br)
            nc.scalar.dma_start(bz_sb[:], bz)
            nc.scalar.dma_start(bh_sb[:], bh)
            for n in range(NT):
                nc.scalar.dma_start(hb_sb[:, n], half(hb, n))
            for m in range(1, MT):
                nc.gpsimd.dma_start(wr_sb[m][:], Wr[m])
            for m in range(MT):
                nc.gpsimd.dma_start(wz_sb[m][:], Wz[m])
            if split:
                for n in range(NT):
                    nc.scalar.dma_start(xb_sb[:, n], half(xb, n))
                for m in range(MT):
                    nc.gpsimd.dma_start(whh_sb[m][:], Whh[m])
                for m in range(MT):
                    nc.gpsimd.dma_start(whx_sb[m][:], Whx[m])
            else:
                for m in range(MT):
                    nc.gpsimd.dma_start(wh_sb[m][:], Wh[m])

            # Gate results, same swizzled layout
            z_sb = gates.tile([P, NT, KO, NFREE], BF16)
            rh_sb = gates.tile([P, NT, KO, NFREE], F8)

            def rz_rhs(kk, n):
                """fp8 moving operand [128,2,512] for concat chunk kk."""
                if kk < KK // 2:
                    return h8_sb[:, n, 2 * kk:2 * kk + 2, :]
                c = kk - KK // 2
                return x8_sb[:, n, 2 * c:2 * c + 2, :]

            def h_rhs(kk, n):
                """fp8 moving operand for the h-gate ([r*h_prev, x])."""
                if kk < KK // 2:
                    return rh_sb[:, n, 2 * kk:2 * kk + 2, :]
                c = kk - KK // 2
                return x8_sb[:, n, 2 * c:2 * c + 2, :]

            def finish(stage, mt, n, ps, width=NFREE, sub=0):
                """PSUM -> activation -> elementwise -> (store)."""
                lo, hi = sub * width, (sub + 1) * width
                if stage == "r":
                    r_tmp = opool.tile([P, width], BF16, tag="rt")
                    nc.scalar.activation(r_tmp, ps, AF.Sigmoid,
                                         bias=br_sb[:, mt:mt + 1],
                                         scale=1.0 / WS)
                    nc.vector.tensor_mul(
                        rh_sb[:, n, mt, lo:hi], r_tmp, hb_sb[:, n, mt, lo:hi])
                elif stage == "z":
                    nc.scalar.activation(z_sb[:, n, mt, lo:hi], ps,
                                         AF.Sigmoid,
                                         bias=bz_sb[:, mt:mt + 1],
                                         scale=1.0 / WS)
                else:  # h = h_prev + z*(h_tilde - h_prev)
                    hpv = hb_sb[:, n, mt, lo:hi]
                    ht = opool.tile([P, width], BF16, tag="ht")
                    nc.scalar.activation(ht, ps, AF.Tanh,
                                         bias=bh_sb[:, mt:mt + 1],
                                         scale=1.0 / WS)
                    nc.vector.tensor_sub(ht, ht, hpv)
                    nc.vector.tensor_mul(ht, ht, z_sb[:, n, mt, lo:hi])
                    nc.vector.tensor_add(ht, ht, hpv)
                    ns = slice(n * NFREE + lo, n * NFREE + hi)
                    nc.sync.dma_start(out[mt * P:(mt + 1) * P, ns], ht)

            def chain(stage, w_sb, rhs, mt, n, nsub=1):
                """One (mt, n) PSUM accumulation chain + its epilogue.

                LDWEIGHTS is emitted 1:1 per matmul by the compiler and at
                ~180ns hides under the ~216ns DR matmul stream, so plain
                k-sequential chains already run at the moving-port roofline;
                chain order only needs to match DMA arrival order.
                """
                ps = ppool.tile([P, NFREE], F32, tag="ps",
                                name=f"ps_{stage}{mt}_{n}")
                for kk in range(KK):
                    nc.tensor.matmul(
                        ps, w_sb[mt][:, 2 * kk:2 * kk + 2, :], rhs(kk, n),
                        start=(kk == 0), stop=(kk == KK - 1), perf_mode=DR)
                w2 = NFREE // nsub
                for s in range(nsub):
                    finish(stage, mt, n, ps[:, s * w2:(s + 1) * w2],
                           width=w2, sub=s)

            for mt in range(MT):
                for n in range(NT):
                    chain("r", wr_sb, rz_rhs, mt, n)
            for mt in range(MT):
                for n in range(NT):
                    chain("z", wz_sb, rz_rhs, mt, n)

            if not split:
                for mt in range(MT):
                    for n in range(NT):
                        last = mt == MT - 1
                        chain("h", wh_sb, h_rhs, mt, n,
                              nsub=4 if (last and n == NT - 1)
                              else (2 if last else 1))
            else:
                for mt in range(MT):
                    for n in range(NT):
                        ps = ppool.tile([P, NFREE], F32, tag="ps",
                                        name=f"ps_h{mt}_{n}")
                        for kc in range(KT // 2):
                            nc.tensor.matmul(
                                ps, whx_sb[mt][:, kc * P:(kc + 1) * P],
                                xb_sb[:, n, kc, :],
                                start=(kc == 0), stop=False)
                        for kk in range(KK // 2):
                            nc.tensor.matmul(
                                ps, whh_sb[mt][:, 2 * kk:2 * kk + 2, :],
                                rh_sb[:, n, 2 * kk:2 * kk + 2, :],
                                start=False, stop=(kk == KK // 2 - 1),
                                perf_mode=DR)
                        last = mt == MT - 1
                        nsub = 4 if (last and n == NT - 1) else (
                            2 if last else 1)
                        w2 = NFREE // nsub
                        for s in range(nsub):
                            finish("h", mt, n, ps[:, s * w2:(s + 1) * w2],
                                   width=w2, sub=s)

    nc.compile()
    return nc


def _prep_inputs(x, h_prev, W_z, b_z, W_r, b_r, W_h, b_h, mode="fp8h"):
    """Host-side relayout: swizzled feature-major acts, m-tiled weights."""
    import ml_dtypes
    F8NP = ml_dtypes.float8_e4m3fn
    BFNP = ml_dtypes.bfloat16
    split = mode == "split"

    def prep_w(W, dt):
        # w[mt, p, ko*128+m] = W[mt*128+m, ko*128+p], scaled for fp8 range
        MTl, Kl = W.shape[0] // P, W.shape[1]
        W4 = (W * WS).reshape(MTl, P, Kl // P, P)      # [mt, m, ko, p]
        return np.ascontiguousarray(
            W4.transpose(0, 3, 2, 1)).reshape(MTl, P, Kl).astype(dt)

    def prep_act(aT, dt):
        # [F, bs] -> [p, n, ko, bw] -> flat [P, AW]
        a4 = aT.reshape(KO, P, NT, NFREE).transpose(1, 2, 0, 3)
        return np.ascontiguousarray(a4).reshape(P, NT * KO * NFREE).astype(dt)

    def prep_b(b):
        return np.ascontiguousarray(b.reshape(MT, P).T)

    xT = np.ascontiguousarray(x.T)                         # [I, B] f32
    hT = np.ascontiguousarray(h_prev.T)                    # [H, B] f32
    shared = {
        "Wr": prep_w(W_r, F8NP), "Wz": prep_w(W_z, F8NP),
        "bz": prep_b(b_z), "br": prep_b(b_r), "bh": prep_b(b_h),
    }
    if split:
        shared["Whh"] = prep_w(W_h[:, :H], F8NP)
        shared["Whx"] = prep_w(W_h[:, H:], BFNP)
    else:
        shared["Wh"] = prep_w(W_h, F8NP)
    in_maps = []
    for c in range(NCORES):
        bs = slice(c * BS, (c + 1) * BS)
        m = dict(shared)
        m["x8"] = prep_act(xT[:, bs], F8NP)
        m["h8"] = prep_act(hT[:, bs], F8NP)
        m["hb"] = prep_act(hT[:, bs], BFNP)
        if split:
            m["xb"] = prep_act(xT[:, bs], BFNP)
        in_maps.append(m)
    return in_maps


def run(inputs, mode="fp8h", trace=False, **run_kwargs):
    """Compile + run on 8 cores. Returns (output [B,H] f32, results)."""
    run_kwargs.pop("mm_dtype", None)
    nc = build_kernel(mode)
    in_maps = _prep_inputs(**inputs, mode=mode)
    res = bass_utils.run_bass_kernel_spmd(
        nc, in_maps, core_ids=list(range(NCORES)), trace=trace, **run_kwargs)
    outT = np.concatenate(
        [res.results[c]["out"] for c in range(NCORES)], axis=1)  # [H, B] bf16
    return np.ascontiguousarray(outT.T).astype(np.float32), res


def kernel(**inputs) -> np.ndarray:
    import time as _time
    try:
        out, _ = run(inputs)
    except Exception:
        # The axon-tunneled device occasionally reports a transient
        # "unrecoverable" state right after a crashed session; a fresh
        # attempt after a short pause recovers.
        _time.sleep(15)
        out, _ = run(inputs)
    return out


# revision 8
# speedup vs baseline: 1.7379x; 1.0003x over previous
"""GRU cell kernel for Trainium2, data-parallel across 8 NeuronCores.

Reference computation (per batch row):
    concat = [h_prev, x]                       # [B, 2048]
    z = sigmoid(concat @ W_z.T + b_z)          # [B, 1024]
    r = sigmoid(concat @ W_r.T + b_r)
    h_tilde = tanh([r*h_prev, x] @ W_h.T + b_h)
    h = (1-z)*h_prev + z*h_tilde

Sharding: batch dim (8192) split 1024/core; weights replicated.
Layout on device is feature-major; batch is the matmul moving dimension,
hidden units the PSUM partition dim. Host transposes in/out.

Matmuls run in fp8-e4m3 with perf_mode=DoubleRow (2 contraction rows per
PE cell). The PE moving port feeds 2 bytes/partition/cycle, so a DR
matmul streams a [256 x 512-batch] contraction chunk in ~512 cycles --
2x the flops of bf16 per cycle; measured ~216ns/MM = ~155 TF/s, the fp8
roofline. Weights are host-scaled by 512 so |w|<=11.3 sits in e4m3's
normal range (raw |w|<=0.022 is subnormal); the activation instruction's
scale operand undoes it for free.

Activations are host-swizzled to [partition, batch-half, feature-chunk,
512] so every DMA moves 4KB-contiguous runs per partition (128
descriptors/transfer instead of 1024 512B ones).

mode:
  fp8h  - all three gates fp8-DR.           (HW rel_fro ~1.76e-2)
  split - r/z fp8-DR; h-gate h-part fp8-DR over r*h_prev, x-part bf16.
                                            (sim rel_fro ~1.25e-2)
"""

import numpy as np

import concourse.bacc as bacc
import concourse.bass as bass
import concourse.mybir as mybir
import concourse.tile as tile
from concourse import bass_utils

P = 128
B = 8192
I = 1024
H = 1024
K = I + H            # 2048 contraction
NCORES = 8
BS = B // NCORES     # 1024 batch rows per core
MT = H // P          # 8 m-tiles (hidden units)
KT = K // P          # 16 k-chunks of 128
KK = K // (2 * P)    # 8 double-chunks of 256 (DoubleRow)
NFREE = 512          # moving free dim (one PSUM bank of fp32)
NT = BS // NFREE     # 2 n-tiles per core
KO = 8               # feature chunks per 1024-feature tensor
WS = 512.0           # host-side weight scale for fp8 range

F32 = mybir.dt.float32
BF16 = mybir.dt.bfloat16
F8 = mybir.dt.float8e4

AF = mybir.ActivationFunctionType
DR = mybir.MatmulPerfMode.DoubleRow


def build_kernel(mode: str = "fp8h"):
    """Build the per-core Bass kernel. Returns compiled nc."""
    assert mode in ("fp8h", "split")
    split = mode == "split"
    nc = bacc.Bacc("TRN2", target_bir_lowering=False, debug=False)

    # DRAM I/O (per-core shapes). Activations are pre-swizzled on the host
    # to [P, NT*KO*NFREE] so each partition's bytes are contiguous.
    AW = NT * KO * NFREE
    x8 = nc.dram_tensor("x8", [P, AW], F8, kind="ExternalInput").ap()
    h8 = nc.dram_tensor("h8", [P, AW], F8, kind="ExternalInput").ap()
    hb = nc.dram_tensor("hb", [P, AW], BF16, kind="ExternalInput").ap()
    Wr = nc.dram_tensor("Wr", [MT, P, K], F8, kind="ExternalInput").ap()
    Wz = nc.dram_tensor("Wz", [MT, P, K], F8, kind="ExternalInput").ap()
    if split:
        xb = nc.dram_tensor("xb", [P, AW], BF16, kind="ExternalInput").ap()
        Whh = nc.dram_tensor("Whh", [MT, P, H], F8, kind="ExternalInput").ap()
        Whx = nc.dram_tensor("Whx", [MT, P, I], BF16,
                             kind="ExternalInput").ap()
    else:
        Wh = nc.dram_tensor("Wh", [MT, P, K], F8, kind="ExternalInput").ap()
    bz = nc.dram_tensor("bz", [P, MT], F32, kind="ExternalInput").ap()
    br = nc.dram_tensor("br", [P, MT], F32, kind="ExternalInput").ap()
    bh = nc.dram_tensor("bh", [P, MT], F32, kind="ExternalInput").ap()
    out = nc.dram_tensor("out", [H, BS], BF16, kind="ExternalOutput").ap()

    with tile.TileContext(nc) as tc:
        with (
            tc.tile_pool(name="acts", bufs=1) as acts,
            tc.tile_pool(name="gates", bufs=1) as gates,
            tc.tile_pool(name="wpool", bufs=1) as wpool,
            tc.tile_pool(name="opool", bufs=10) as opool,
            tc.tile_pool(name="ppool", bufs=8, space="PSUM") as ppool,
        ):
            bz_sb = acts.tile([P, MT], F32)
            br_sb = acts.tile([P, MT], F32)
            bh_sb = acts.tile([P, MT], F32)

            # Weight tiles, [P, KT, P]: [:, 2k:2k+2, :] is a DoubleRow
            # stationary operand [128, 2, 128].
            wr_sb = [wpool.tile([P, KT, P], F8, name=f"wr{m}")
                     for m in range(MT)]
            wz_sb = [wpool.tile([P, KT, P], F8, name=f"wz{m}")
                     for m in range(MT)]
            if split:
                whh_sb = [wpool.tile([P, KT // 2, P], F8, name=f"whh{m}")
                          for m in range(MT)]
                whx_sb = [wpool.tile([P, I], BF16, name=f"whx{m}")
                          for m in range(MT)]
            else:
                wh_sb = [wpool.tile([P, KT, P], F8, name=f"wh{m}")
                         for m in range(MT)]

            # Pre-warm the ACT sigmoid table during the DMA fill.
            warm = acts.tile([P, 1], F32)
            nc.vector.memset(warm[:], 0.0)
            nc.scalar.activation(warm[:], warm[:], AF.Sigmoid)

            # Persistent activations: [p, n-half, ko, bw]
            x8_sb = acts.tile([P, NT, KO, NFREE], F8)
            h8_sb = acts.tile([P, NT, KO, NFREE], F8)
            hb_sb = acts.tile([P, NT, KO, NFREE], BF16)
            xb_sb = (acts.tile([P, NT, KO, NFREE], BF16, name="xb_sb")
                     if split else None)

            def half(dram, n):
                return dram[:, n * KO * NFREE:(n + 1) * KO * NFREE]

            # Head DMA schedule. The h-half of the acts rides the sync ring
            # and the x-half the scalar ring, so the mt0 chains' kk0-3
            # (h_prev) and kk4-7 (x) payloads stream in parallel instead of
            # serializing on one HWDGE ring (the early window is chip-HBM
            # contended -- all 8 cores load at once). The gpsimd SWDGE queue
            # (~1us extra latency, otherwise idle) takes everything needed
            # later than ~15us: remaining weights, then hb, then wz/wh.
            q0 = 2 * NFREE  # first two feature-chunks of a half
            nc.scalar.dma_start(wr_sb[0][:], Wr[0])
            nc.sync.dma_start(h8_sb[:, 0, 0:2, :], h8[:, 0:q0])
            nc.sync.dma_start(h8_sb[:, 0, 2:, :], h8[:, q0:KO * NFREE])
            nc.sync.dma_start(h8_sb[:, 1], half(h8, 1))
            nc.scalar.dma_start(x8_sb[:, 0], half(x8, 0))
            nc.scalar.dma_start(x8_sb[:, 1], half(x8, 1))
            nc.scalar.dma_start(br_sb[:], br)
            nc.scalar.dma_start(bz_sb[:], bz)
            nc.scalar.dma_start(bh_sb[:], bh)
            for m in range(1, MT):
                nc.gpsimd.dma_start(wr_sb[m][:], Wr[m])
            for n in range(NT):
                nc.gpsimd.dma_start(hb_sb[:, n], half(hb, n))
            for m in range(MT):
                nc.gpsimd.dma_start(wz_sb[m][:], Wz[m])
            if split:
                for n in range(NT):
                    nc.scalar.dma_start(xb_sb[:, n], half(xb, n))
                for m in range(MT):
                    nc.gpsimd.dma_start(whh_sb[m][:], Whh[m])
                for m in range(MT):
                    nc.gpsimd.dma_start(whx_sb[m][:], Whx[m])
            else:
                for m in range(MT):
                    nc.gpsimd.dma_start(wh_sb[m][:], Wh[m])

            # Gate results, same swizzled layout
            z_sb = gates.tile([P, NT, KO, NFREE], BF16)
            rh_sb = gates.tile([P, NT, KO, NFREE], F8)

            def rz_rhs(kk, n):
                """fp8 moving operand [128,2,512] for concat chunk kk."""
                if kk < KK // 2:
                    return h8_sb[:, n, 2 * kk:2 * kk + 2, :]
                c = kk - KK // 2
                return x8_sb[:, n, 2 * c:2 * c + 2, :]

            def h_rhs(kk, n):
                """fp8 moving operand for the h-gate ([r*h_prev, x])."""
                if kk < KK // 2:
                    return rh_sb[:, n, 2 * kk:2 * kk + 2, :]
                c = kk - KK // 2
                return x8_sb[:, n, 2 * c:2 * c + 2, :]

            def finish(stage, mt, n, ps, width=NFREE, sub=0):
                """PSUM -> activation -> elementwise -> (store)."""
                lo, hi = sub * width, (sub + 1) * width
                if stage == "r":
                    r_tmp = opool.tile([P, width], BF16, tag="rt")
                    nc.scalar.activation(r_tmp, ps, AF.Sigmoid,
                                         bias=br_sb[:, mt:mt + 1],
                                         scale=1.0 / WS)
                    nc.vector.tensor_mul(
                        rh_sb[:, n, mt, lo:hi], r_tmp, hb_sb[:, n, mt, lo:hi])
                elif stage == "z":
                    nc.scalar.activation(z_sb[:, n, mt, lo:hi], ps,
                                         AF.Sigmoid,
                                         bias=bz_sb[:, mt:mt + 1],
                                         scale=1.0 / WS)
                else:  # h = h_prev + z*(h_tilde - h_prev)
                    hpv = hb_sb[:, n, mt, lo:hi]
                    ht = opool.tile([P, width], BF16, tag="ht")
                    nc.scalar.activation(ht, ps, AF.Tanh,
                                         bias=bh_sb[:, mt:mt + 1],
                                         scale=1.0 / WS)
                    nc.vector.tensor_sub(ht, ht, hpv)
                    nc.vector.tensor_mul(ht, ht, z_sb[:, n, mt, lo:hi])
                    nc.vector.tensor_add(ht, ht, hpv)
                    ns = slice(n * NFREE + lo, n * NFREE + hi)
                    nc.sync.dma_start(out[mt * P:(mt + 1) * P, ns], ht)

            def chain(stage, w_sb, rhs, mt, n, nsub=1, nchain=1):
                """One (mt, n) PSUM accumulation chain + its epilogue.

                LDWEIGHTS is emitted 1:1 per matmul by the compiler and at
                ~135ns hides under the ~216ns moving-port-bound DR matmul
                stream, so plain k-sequential chains already run at the
                roofline; chain order only needs to match DMA arrival order.
                nchain>1 splits the matmuls into narrower column chains so
                the epilogue of chain c pipelines under chain c+1's matmuls
                (used for the very last group to shrink the kernel tail).
                """
                wc = NFREE // nchain
                for c in range(nchain):
                    ps = ppool.tile([P, wc], F32, tag="ps",
                                    name=f"ps_{stage}{mt}_{n}_{c}")
                    for kk in range(KK):
                        nc.tensor.matmul(
                            ps, w_sb[mt][:, 2 * kk:2 * kk + 2, :],
                            rhs(kk, n)[:, :, c * wc:(c + 1) * wc],
                            start=(kk == 0), stop=(kk == KK - 1),
                            perf_mode=DR)
                    w2 = wc // nsub
                    for s in range(nsub):
                        finish(stage, mt, n, ps[:, s * w2:(s + 1) * w2],
                               width=w2, sub=c * nsub + s)

            for mt in range(MT):
                for n in range(NT):
                    chain("r", wr_sb, rz_rhs, mt, n)
            for mt in range(MT):
                for n in range(NT):
                    chain("z", wz_sb, rz_rhs, mt, n)

            if not split:
                for mt in range(MT):
                    for n in range(NT):
                        last = mt == MT - 1 and n == NT - 1
                        chain("h", wh_sb, h_rhs, mt, n,
                              nsub=2 if mt == MT - 1 else 1,
                              nchain=2 if last else 1)
            else:
                for mt in range(MT):
                    for n in range(NT):
                        ps = ppool.tile([P, NFREE], F32, tag="ps",
                                        name=f"ps_h{mt}_{n}")
                        for kc in range(KT // 2):
                            nc.tensor.matmul(
                                ps, whx_sb[mt][:, kc * P:(kc + 1) * P],
                                xb_sb[:, n, kc, :],
                                start=(kc == 0), stop=False)
                        for kk in range(KK // 2):
                            nc.tensor.matmul(
                                ps, whh_sb[mt][:, 2 * kk:2 * kk + 2, :],
                                rh_sb[:, n, 2 * kk:2 * kk + 2, :],
                                start=False, stop=(kk == KK // 2 - 1),
                                perf_mode=DR)
                        last = mt == MT - 1
                        nsub = 4 if (last and n == NT - 1) else (
                            2 if last else 1)
                        w2 = NFREE // nsub
                        for s in range(nsub):
                            finish("h", mt, n, ps[:, s * w2:(s + 1) * w2],
                                   width=w2, sub=s)

    nc.compile()
    return nc


def _prep_inputs(x, h_prev, W_z, b_z, W_r, b_r, W_h, b_h, mode="fp8h"):
    """Host-side relayout: swizzled feature-major acts, m-tiled weights."""
    import ml_dtypes
    F8NP = ml_dtypes.float8_e4m3fn
    BFNP = ml_dtypes.bfloat16
    split = mode == "split"

    def prep_w(W, dt):
        # w[mt, p, ko*128+m] = W[mt*128+m, ko*128+p], scaled for fp8 range
        MTl, Kl = W.shape[0] // P, W.shape[1]
        W4 = (W * WS).reshape(MTl, P, Kl // P, P)      # [mt, m, ko, p]
        return np.ascontiguousarray(
            W4.transpose(0, 3, 2, 1)).reshape(MTl, P, Kl).astype(dt)

    def prep_act(aT, dt):
        # [F, bs] -> [p, n, ko, bw] -> flat [P, AW]
        a4 = aT.reshape(KO, P, NT, NFREE).transpose(1, 2, 0, 3)
        return np.ascontiguousarray(a4).reshape(P, NT * KO * NFREE).astype(dt)

    def prep_b(b):
        return np.ascontiguousarray(b.reshape(MT, P).T)

    xT = np.ascontiguousarray(x.T)                         # [I, B] f32
    hT = np.ascontiguousarray(h_prev.T)                    # [H, B] f32
    shared = {
        "Wr": prep_w(W_r, F8NP), "Wz": prep_w(W_z, F8NP),
        "bz": prep_b(b_z), "br": prep_b(b_r), "bh": prep_b(b_h),
    }
    if split:
        shared["Whh"] = prep_w(W_h[:, :H], F8NP)
        shared["Whx"] = prep_w(W_h[:, H:], BFNP)
    else:
        shared["Wh"] = prep_w(W_h, F8NP)
    in_maps = []
    for c in range(NCORES):
        bs = slice(c * BS, (c + 1) * BS)
        m = dict(shared)
        m["x8"] = prep_act(xT[:, bs], F8NP)
        m["h8"] = prep_act(hT[:, bs], F8NP)
        m["hb"] = prep_act(hT[:, bs], BFNP)
        if split:
            m["xb"] = prep_act(xT[:, bs], BFNP)
        in_maps.append(m)
    return in_maps


def run(inputs, mode="fp8h", trace=False, **run_kwargs):
    """Compile + run on 8 cores. Returns (output [B,H] f32, results)."""
    run_kwargs.pop("mm_dtype", None)
    nc = build_kernel(mode)
    in_maps = _prep_inputs(**inputs, mode=mode)
    res = bass_utils.run_bass_kernel_spmd(
        nc, in_maps, core_ids=list(range(NCORES)), trace=trace, **run_kwargs)
    outT = np.concatenate(
        [res.results[c]["out"] for c in range(NCORES)], axis=1)  # [H, B] bf16
    return np.ascontiguousarray(outT.T).astype(np.float32), res


def kernel(**inputs) -> np.ndarray:
    import time as _time
    try:
        out, _ = run(inputs)
    except Exception:
        # The axon-tunneled device occasionally reports a transient
        # "unrecoverable" state right after a crashed session; a fresh
        # attempt after a short pause recovers.
        _time.sleep(15)
        out, _ = run(inputs)
    return out


# revision 10
# speedup vs baseline: 1.7718x; 1.0195x over previous
"""GRU cell kernel for Trainium2, data-parallel across 8 NeuronCores.

Reference computation (per batch row):
    concat = [h_prev, x]                       # [B, 2048]
    z = sigmoid(concat @ W_z.T + b_z)          # [B, 1024]
    r = sigmoid(concat @ W_r.T + b_r)
    h_tilde = tanh([r*h_prev, x] @ W_h.T + b_h)
    h = (1-z)*h_prev + z*h_tilde

Sharding: batch dim (8192) split 1024/core; weights replicated.
Layout on device is feature-major; batch is the matmul moving dimension,
hidden units the PSUM partition dim. Host transposes in/out.

Matmuls run in fp8-e4m3 with perf_mode=DoubleRow (2 contraction rows per
PE cell). The PE moving port feeds 2 bytes/partition/cycle, so a DR
matmul streams a [256 x 512-batch] contraction chunk in ~512 cycles --
2x the flops of bf16 per cycle; measured ~216ns/MM = ~155 TF/s, the fp8
roofline. Weights are host-scaled by 512 so |w|<=11.3 sits in e4m3's
normal range (raw |w|<=0.022 is subnormal); the activation instruction's
scale operand undoes it for free.

Activations are host-swizzled to [partition, batch-half, feature-chunk,
512] so every DMA moves 4KB-contiguous runs per partition (128
descriptors/transfer instead of 1024 512B ones).

mode:
  fp8h  - all three gates fp8-DR.           (HW rel_fro ~1.76e-2)
  split - r/z fp8-DR; h-gate h-part fp8-DR over r*h_prev, x-part bf16.
                                            (sim rel_fro ~1.25e-2)
"""

import numpy as np

import concourse.bacc as bacc
import concourse.bass as bass
import concourse.mybir as mybir
import concourse.tile as tile
from concourse import bass_utils

P = 128
B = 8192
I = 1024
H = 1024
K = I + H            # 2048 contraction
NCORES = 8
BS = B // NCORES     # 1024 batch rows per core
MT = H // P          # 8 m-tiles (hidden units)
KT = K // P          # 16 k-chunks of 128
KK = K // (2 * P)    # 8 double-chunks of 256 (DoubleRow)
NFREE = 512          # moving free dim (one PSUM bank of fp32)
NT = BS // NFREE     # 2 n-tiles per core
KO = 8               # feature chunks per 1024-feature tensor
WS = 512.0           # host-side weight scale for fp8 range

F32 = mybir.dt.float32
BF16 = mybir.dt.bfloat16
F8 = mybir.dt.float8e4

AF = mybir.ActivationFunctionType
DR = mybir.MatmulPerfMode.DoubleRow


def build_kernel(mode: str = "fp8h"):
    """Build the per-core Bass kernel. Returns compiled nc."""
    assert mode in ("fp8h", "split")
    split = mode == "split"
    nc = bacc.Bacc("TRN2", target_bir_lowering=False, debug=False)

    # DRAM I/O (per-core shapes). Activations are pre-swizzled on the host
    # to [P, NT*KO*NFREE] so each partition's bytes are contiguous.
    AW = NT * KO * NFREE
    x8 = nc.dram_tensor("x8", [P, AW], F8, kind="ExternalInput").ap()
    h8 = nc.dram_tensor("h8", [P, AW], F8, kind="ExternalInput").ap()
    hb = nc.dram_tensor("hb", [P, AW], BF16, kind="ExternalInput").ap()
    Wr = nc.dram_tensor("Wr", [MT, P, K], F8, kind="ExternalInput").ap()
    Wz = nc.dram_tensor("Wz", [MT, P, K], F8, kind="ExternalInput").ap()
    if split:
        xb = nc.dram_tensor("xb", [P, AW], BF16, kind="ExternalInput").ap()
        Whh = nc.dram_tensor("Whh", [MT, P, H], F8, kind="ExternalInput").ap()
        Whx = nc.dram_tensor("Whx", [MT, P, I], BF16,
                             kind="ExternalInput").ap()
    else:
        Wh = nc.dram_tensor("Wh", [MT, P, K], F8, kind="ExternalInput").ap()
    bz = nc.dram_tensor("bz", [P, MT], F32, kind="ExternalInput").ap()
    br = nc.dram_tensor("br", [P, MT], F32, kind="ExternalInput").ap()
    bh = nc.dram_tensor("bh", [P, MT], F32, kind="ExternalInput").ap()
    out = nc.dram_tensor("out", [H, BS], BF16, kind="ExternalOutput").ap()

    with tile.TileContext(nc) as tc:
        with (
            tc.tile_pool(name="acts", bufs=1) as acts,
            tc.tile_pool(name="gates", bufs=1) as gates,
            tc.tile_pool(name="wpool", bufs=1) as wpool,
            tc.tile_pool(name="opool", bufs=10) as opool,
            tc.tile_pool(name="ppool", bufs=8, space="PSUM") as ppool,
        ):
            bz_sb = acts.tile([P, MT], F32)
            br_sb = acts.tile([P, MT], F32)
            bh_sb = acts.tile([P, MT], F32)

            # Weight tiles, [P, KT, P]: [:, 2k:2k+2, :] is a DoubleRow
            # stationary operand [128, 2, 128].
            wr_sb = [wpool.tile([P, KT, P], F8, name=f"wr{m}")
                     for m in range(MT)]
            wz_sb = [wpool.tile([P, KT, P], F8, name=f"wz{m}")
                     for m in range(MT)]
            if split:
                whh_sb = [wpool.tile([P, KT // 2, P], F8, name=f"whh{m}")
                          for m in range(MT)]
                whx_sb = [wpool.tile([P, I], BF16, name=f"whx{m}")
                          for m in range(MT)]
            else:
                wh_sb = [wpool.tile([P, KT, P], F8, name=f"wh{m}")
                         for m in range(MT)]

            # Pre-warm the ACT sigmoid table during the DMA fill.
            warm = acts.tile([P, 1], F32)
            nc.vector.memset(warm[:], 0.0)
            nc.scalar.activation(warm[:], warm[:], AF.Sigmoid)

            # Persistent activations: [p, n-half, ko, bw]
            x8_sb = acts.tile([P, NT, KO, NFREE], F8)
            h8_sb = acts.tile([P, NT, KO, NFREE], F8)
            hb_sb = acts.tile([P, NT, KO, NFREE], BF16)
            xb_sb = (acts.tile([P, NT, KO, NFREE], BF16, name="xb_sb")
                     if split else None)

            def half(dram, n):
                return dram[:, n * KO * NFREE:(n + 1) * KO * NFREE]

            # Head DMA schedule. The h-half of the acts rides the sync ring
            # and the x-half the scalar ring, so the mt0 chains' kk0-3
            # (h_prev) and kk4-7 (x) payloads stream in parallel instead of
            # serializing on one HWDGE ring (the early window is chip-HBM
            # contended -- all 8 cores load at once). The gpsimd SWDGE queue
            # (~1us extra latency, otherwise idle) takes everything needed
            # later than ~15us: remaining weights, then hb, then wz/wh.
            q0 = 2 * NFREE  # first two feature-chunks of a half
            nc.scalar.dma_start(wr_sb[0][:], Wr[0])
            nc.sync.dma_start(h8_sb[:, 0, 0:2, :], h8[:, 0:q0])
            nc.sync.dma_start(h8_sb[:, 0, 2:, :], h8[:, q0:KO * NFREE])
            nc.sync.dma_start(h8_sb[:, 1], half(h8, 1))
            nc.scalar.dma_start(br_sb[:], br)
            nc.scalar.dma_start(x8_sb[:, 0], half(x8, 0))
            nc.scalar.dma_start(x8_sb[:, 1], half(x8, 1))
            nc.scalar.dma_start(bz_sb[:], bz)
            nc.scalar.dma_start(bh_sb[:], bh)
            # wr1 is needed ~2us after the first chain; everything after it
            # is bulk. SDMA engines round-robin between queues at packet
            # granularity, so un-gated bulk on the gpsimd ring would steal
            # ~1/3 of the chip-contended head bandwidth from the critical
            # act loads above. The dummy copy below reads from the x8 n1
            # half, so the tile framework makes the bulk descriptors wait
            # until the last critical act DMA has landed.
            nc.gpsimd.dma_start(wr_sb[1][:], Wr[1])
            dma_gate = opool.tile([P, 8], F8, name="dma_gate")
            nc.gpsimd.tensor_copy(dma_gate[:], x8_sb[:, 1, 0, 0:8])
            for m in range(2, MT):
                nc.gpsimd.dma_start(wr_sb[m][:], Wr[m])
            for n in range(NT):
                nc.gpsimd.dma_start(hb_sb[:, n], half(hb, n))
            for m in range(MT):
                nc.gpsimd.dma_start(wz_sb[m][:], Wz[m])
            if split:
                for n in range(NT):
                    nc.scalar.dma_start(xb_sb[:, n], half(xb, n))
                for m in range(MT):
                    nc.gpsimd.dma_start(whh_sb[m][:], Whh[m])
                for m in range(MT):
                    nc.gpsimd.dma_start(whx_sb[m][:], Whx[m])
            else:
                for m in range(MT):
                    nc.gpsimd.dma_start(wh_sb[m][:], Wh[m])

            # Gate results, same swizzled layout
            z_sb = gates.tile([P, NT, KO, NFREE], BF16)
            rh_sb = gates.tile([P, NT, KO, NFREE], F8)

            def rz_rhs(kk, n):
                """fp8 moving operand [128,2,512] for concat chunk kk."""
                if kk < KK // 2:
                    return h8_sb[:, n, 2 * kk:2 * kk + 2, :]
                c = kk - KK // 2
                return x8_sb[:, n, 2 * c:2 * c + 2, :]

            def h_rhs(kk, n):
                """fp8 moving operand for the h-gate ([r*h_prev, x])."""
                if kk < KK // 2:
                    return rh_sb[:, n, 2 * kk:2 * kk + 2, :]
                c = kk - KK // 2
                return x8_sb[:, n, 2 * c:2 * c + 2, :]

            def finish(stage, mt, n, ps, width=NFREE, sub=0):
                """PSUM -> activation -> elementwise -> (store)."""
                lo, hi = sub * width, (sub + 1) * width
                if stage == "r":
                    r_tmp = opool.tile([P, width], BF16, tag="rt")
                    nc.scalar.activation(r_tmp, ps, AF.Sigmoid,
                                         bias=br_sb[:, mt:mt + 1],
                                         scale=1.0 / WS)
                    nc.vector.tensor_mul(
                        rh_sb[:, n, mt, lo:hi], r_tmp, hb_sb[:, n, mt, lo:hi])
                elif stage == "z":
                    nc.scalar.activation(z_sb[:, n, mt, lo:hi], ps,
                                         AF.Sigmoid,
                                         bias=bz_sb[:, mt:mt + 1],
                                         scale=1.0 / WS)
                else:  # h = h_prev + z*(h_tilde - h_prev)
                    hpv = hb_sb[:, n, mt, lo:hi]
                    ht = opool.tile([P, width], BF16, tag="ht")
                    nc.scalar.activation(ht, ps, AF.Tanh,
                                         bias=bh_sb[:, mt:mt + 1],
                                         scale=1.0 / WS)
                    nc.vector.tensor_sub(ht, ht, hpv)
                    nc.vector.tensor_mul(ht, ht, z_sb[:, n, mt, lo:hi])
                    nc.vector.tensor_add(ht, ht, hpv)
                    ns = slice(n * NFREE + lo, n * NFREE + hi)
                    nc.sync.dma_start(out[mt * P:(mt + 1) * P, ns], ht)

            def chain(stage, w_sb, rhs, mt, n, nsub=1, nchain=1):
                """One (mt, n) PSUM accumulation chain + its epilogue.

                LDWEIGHTS is emitted 1:1 per matmul by the compiler and at
                ~135ns hides under the ~216ns moving-port-bound DR matmul
                stream, so plain k-sequential chains already run at the
                roofline; chain order only needs to match DMA arrival order.
                nchain>1 splits the matmuls into narrower column chains so
                the epilogue of chain c pipelines under chain c+1's matmuls
                (used for the very last group to shrink the kernel tail).
                """
                wc = NFREE // nchain
                for c in range(nchain):
                    ps = ppool.tile([P, wc], F32, tag="ps",
                                    name=f"ps_{stage}{mt}_{n}_{c}")
                    for kk in range(KK):
                        nc.tensor.matmul(
                            ps, w_sb[mt][:, 2 * kk:2 * kk + 2, :],
                            rhs(kk, n)[:, :, c * wc:(c + 1) * wc],
                            start=(kk == 0), stop=(kk == KK - 1),
                            perf_mode=DR)
                    w2 = wc // nsub
                    for s in range(nsub):
                        finish(stage, mt, n, ps[:, s * w2:(s + 1) * w2],
                               width=w2, sub=c * nsub + s)

            for mt in range(MT):
                for n in range(NT):
                    chain("r", wr_sb, rz_rhs, mt, n)
            for mt in range(MT):
                for n in range(NT):
                    chain("z", wz_sb, rz_rhs, mt, n)

            if not split:
                for mt in range(MT):
                    for n in range(NT):
                        last = mt == MT - 1 and n == NT - 1
                        chain("h", wh_sb, h_rhs, mt, n,
                              nsub=1 if last else (2 if mt == MT - 1 else 1),
                              nchain=2 if last else 1)
            else:
                for mt in range(MT):
                    for n in range(NT):
                        ps = ppool.tile([P, NFREE], F32, tag="ps",
                                        name=f"ps_h{mt}_{n}")
                        for kc in range(KT // 2):
                            nc.tensor.matmul(
                                ps, whx_sb[mt][:, kc * P:(kc + 1) * P],
                                xb_sb[:, n, kc, :],
                                start=(kc == 0), stop=False)
                        for kk in range(KK // 2):
                            nc.tensor.matmul(
                                ps, whh_sb[mt][:, 2 * kk:2 * kk + 2, :],
                                rh_sb[:, n, 2 * kk:2 * kk + 2, :],
                                start=False, stop=(kk == KK // 2 - 1),
                                perf_mode=DR)
                        last = mt == MT - 1
                        nsub = 4 if (last and n == NT - 1) else (
                            2 if last else 1)
                        w2 = NFREE // nsub
                        for s in range(nsub):
                            finish("h", mt, n, ps[:, s * w2:(s + 1) * w2],
                                   width=w2, sub=s)

    nc.compile()
    return nc


def _prep_inputs(x, h_prev, W_z, b_z, W_r, b_r, W_h, b_h, mode="fp8h"):
    """Host-side relayout: swizzled feature-major acts, m-tiled weights."""
    import ml_dtypes
    F8NP = ml_dtypes.float8_e4m3fn
    BFNP = ml_dtypes.bfloat16
    split = mode == "split"

    def prep_w(W, dt):
        # w[mt, p, ko*128+m] = W[mt*128+m, ko*128+p], scaled for fp8 range
        MTl, Kl = W.shape[0] // P, W.shape[1]
        W4 = (W * WS).reshape(MTl, P, Kl // P, P)      # [mt, m, ko, p]
        return np.ascontiguousarray(
            W4.transpose(0, 3, 2, 1)).reshape(MTl, P, Kl).astype(dt)

    def prep_act(aT, dt):
        # [F, bs] -> [p, n, ko, bw] -> flat [P, AW]
        a4 = aT.reshape(KO, P, NT, NFREE).transpose(1, 2, 0, 3)
        return np.ascontiguousarray(a4).reshape(P, NT * KO * NFREE).astype(dt)

    def prep_b(b):
        return np.ascontiguousarray(b.reshape(MT, P).T)

    xT = np.ascontiguousarray(x.T)                         # [I, B] f32
    hT = np.ascontiguousarray(h_prev.T)                    # [H, B] f32
    shared = {
        "Wr": prep_w(W_r, F8NP), "Wz": prep_w(W_z, F8NP),
        "bz": prep_b(b_z), "br": prep_b(b_r), "bh": prep_b(b_h),
    }
    if split:
        shared["Whh"] = prep_w(W_h[:, :H], F8NP)
        shared["Whx"] = prep_w(W_h[:, H:], BFNP)
    else:
        shared["Wh"] = prep_w(W_h, F8NP)
    in_maps = []
    for c in range(NCORES):
        bs = slice(c * BS, (c + 1) * BS)
        m = dict(shared)
        m["x8"] = prep_act(xT[:, bs], F8NP)
        m["h8"] = prep_act(hT[:, bs], F8NP)
        m["hb"] = prep_act(hT[:, bs], BFNP)
        if split:
            m["xb"] = prep_act(xT[:, bs], BFNP)
        in_maps.append(m)
    return in_maps


def run(inputs, mode="fp8h", trace=False, **run_kwargs):
    """Compile + run on 8 cores. Returns (output [B,H] f32, results)."""
    run_kwargs.pop("mm_dtype", None)
    nc = build_kernel(mode)
    in_maps = _prep_inputs(**inputs, mode=mode)
    res = bass_utils.run_bass_kernel_spmd(
        nc, in_maps, core_ids=list(range(NCORES)), trace=trace, **run_kwargs)
    outT = np.concatenate(
        [res.results[c]["out"] for c in range(NCORES)], axis=1)  # [H, B] bf16
    return np.ascontiguousarray(outT.T).astype(np.float32), res


def kernel(**inputs) -> np.ndarray:
    import time as _time
    try:
        out, _ = run(inputs)
    except Exception:
        # The axon-tunneled device occasionally reports a transient
        # "unrecoverable" state right after a crashed session; a fresh
        # attempt after a short pause recovers.
        _time.sleep(15)
        out, _ = run(inputs)
    return out
